# revision 10
# baseline (speedup 1.0000x reference)
"""Trainium2 Bass kernel for a single-head unscaled-softmax attention layer.

Reference computation (fp32):
    q = X @ Wq + bq ; k = X @ Wk + bk ; v = X @ Wv + bv        X: [B=4, N=2048, D=1024]
    out = softmax(q @ k^T, axis=-1) @ v                         (no 1/sqrt(d) scale)

Sharding: 8 cores = (batch b, sequence half h). Each core owns 1024 query
rows; it projects Q/K/V only for its OWN rows, then core pairs exchange
their K^T/V halves with a 2-core AllGather so each core attends over the
full 2048 keys of its batch (flash-style seq-block parallel - the single-
head softmax couples the full feature dim, so tensor parallel is worse).
Attention is permutation-invariant over keys, so the gathered
[even-core rows, odd-core rows] key order needs no correction; the kernel
is identical SPMD across all 8 cores.

Per-core kernel (matmuls in float32r - fp32 storage, fast PE mode):
  phase K: K^T[e,m_own] = Wk^T X^T -> DRAM, pair AllGather -> K^T full
  phase Q: Q^T[e,n]     = Wq^T X^T -> SBUF resident
  phase V: V[m_own,e]   = X Wv     -> DRAM, pair AllGather -> V full -> SBUF
  attention (2 halves of 512 query rows):
    S^T[m,n] = K Q^T      (psum, contract e over K^T/Q^T chunks)
    P^T      = exp(S^T)   (ACT, no max subtraction - logits < ~60 so exp
                           stays in fp32 range; softmax is shift-invariant)
    out[n,e] = P^T.T @ V  (contract m over all 16 m-chunks in psum)
    den[n]   = P^T.T @ 1  (rides the same stationary weights)
    out     /= den        (DVE reciprocal + per-partition scale)
"""

import numpy as np

import concourse.bass as bass
import concourse.mybir as mybir
import concourse.tile as tile

B, N, D = 4, 2048, 1024
NCORES = 8
P = 128
NQ = N // 2          # query rows per core
KD = D // P          # 8 contraction chunks over d_in
KE = D // P          # 8 chunks over d_out (e)
MC = N // P          # 16 key chunks of 128
MB = N // 512        # 4 key blocks of 512
FP = mybir.dt.float32
FPR = mybir.dt.float32r


def _split_sync_waits(nc, max_waits=1):
    """Walrus codegen on this container accepts at most one sync-wait command
    per instruction; hoist excess waits onto NoOps injected just before the
    instruction on the same engine (engines execute in order, so blocking at
    the NoOp is equivalent)."""
    mb = mybir
    for fn in nc.m.functions:
        for bb in fn.blocks:
            insts = list(bb.instructions)
            new = []
            changed = False
            for inst in insts:
                si = getattr(inst, "sync_info", None)
                if si is not None and si.on_wait and len(si.on_wait) > max_waits:
                    waits = list(si.on_wait)
                    keep = waits[-max_waits:]
                    excess = waits[:-max_waits]
                    for i in range(0, len(excess), max_waits):
                        chunk = excess[i : i + max_waits]
                        nop = mb.InstNoOp(
                            name=f"{inst.name}-sw{i}", ins=[], outs=[],
                            engine=inst.engine,
                        )
                        nop.sync_info = mb.SyncInfo(on_wait=chunk, on_update=[])
                        new.append(nop)
                    inst.sync_info = mb.SyncInfo(
                        on_wait=keep, on_update=list(si.on_update or [])
                    )
                    changed = True
                new.append(inst)
            if changed:
                bb.instructions = new


def _emit_body(nc, tc, rep, params, consts, pools):
    """One full attention computation for this core's shard."""
    XT, Wq, Wk, Wv, OUT = params
    bq_t, bk_t, bv_row, ones_col, ones_row = consts
    v_pool, qt_pool, mm_ps, st_ps, out_ps = pools
    MM = nc.tensor.matmul
    PAIRS = [[0, 1], [2, 3], [4, 5], [6, 7]]

    ktown = nc.dram_tensor(f"ktown{rep}", [D, NQ], FPR)
    ktfull = nc.dram_tensor(f"ktfull{rep}", [2 * D, NQ], FPR)
    vown = nc.dram_tensor(f"vown{rep}", [NQ, D], FPR)
    vfull = nc.dram_tensor(f"vfull{rep}", [N, D], FPR)
    vt = [v_pool.tile([P, D], FPR, name=f"vt{rep}_{i}", tag="vt") for i in range(MC)]
    qt = [qt_pool.tile([P, NQ], FPR, name=f"qt{rep}_{i}", tag="qt") for i in range(KE)]

    # ---------------- projections ----------------
    with (
        tc.tile_pool(name=f"w{rep}", bufs=KD) as w_pool,
        tc.tile_pool(name=f"xt{rep}", bufs=10) as xt_pool,
        tc.tile_pool(name=f"kst{rep}", bufs=3) as kst_pool,
    ):
        # K phase: K^T[e, m_own] -> DRAM, then pair AllGather
        wk = [w_pool.tile([P, D], FPR, name=f"wk{rep}_{i}", tag="w") for i in range(KD)]
        for d in range(KD):
            nc.gpsimd.dma_start(wk[d][:], Wk[d * P : (d + 1) * P, :])
        for mb in range(NQ // 512):
            xts = [xt_pool.tile([P, 512], FPR, name=f"xtk{rep}_{i}", tag="xt")
                   for i in range(KD)]
            for d in range(KD):
                nc.gpsimd.dma_start(
                    xts[d][:], XT[d * P : (d + 1) * P, mb * 512 : (mb + 1) * 512]
                )
            for e in range(KE):
                ps = mm_ps.tile([P, 512], FP, name="mm", tag="mm")
                for d in range(KD):
                    MM(ps[:], wk[d][:, e * P : (e + 1) * P], xts[d][:],
                       start=(d == 0), stop=(d == KD - 1))
                st = kst_pool.tile([P, 512], FPR, name="kst", tag="kst")
                nc.vector.tensor_scalar_add(st[:], ps[:], bk_t[:, e : e + 1])
                nc.gpsimd.dma_start(
                    ktown[e * P : (e + 1) * P, mb * 512 : (mb + 1) * 512], st[:]
                )
        nc.gpsimd.collective_compute(
            "AllGather", mybir.AluOpType.bypass, replica_groups=PAIRS,
            ins=[ktown[:]], outs=[ktfull[:]],
        )

        # Q phase: Q^T[e, n] -> SBUF (query rows are XT columns 0..NQ)
        wq = [w_pool.tile([P, D], FPR, name=f"wq{rep}_{i}", tag="w") for i in range(KD)]
        for d in range(KD):
            nc.gpsimd.dma_start(wq[d][:], Wq[d * P : (d + 1) * P, :])
        for mb in range(NQ // 512):
            xts = [xt_pool.tile([P, 512], FPR, name=f"xtq{rep}_{i}", tag="xt")
                   for i in range(KD)]
            for d in range(KD):
                nc.gpsimd.dma_start(
                    xts[d][:], XT[d * P : (d + 1) * P, mb * 512 : (mb + 1) * 512]
                )
            for e in range(KE):
                ps = mm_ps.tile([P, 512], FP, name="mm", tag="mm")
                for d in range(KD):
                    MM(ps[:], wq[d][:, e * P : (e + 1) * P], xts[d][:],
                       start=(d == 0), stop=(d == KD - 1))
                nc.vector.tensor_scalar_add(
                    qt[e][:, mb * 512 : (mb + 1) * 512], ps[:], bq_t[:, e : e + 1]
                )

        # V phase: V[m_own, e] -> DRAM, then pair AllGather -> SBUF tiles
        wv = [w_pool.tile([P, D], FPR, name=f"wv{rep}_{i}", tag="w") for i in range(KD)]
        for d in range(KD):
            nc.gpsimd.dma_start(wv[d][:], Wv[d * P : (d + 1) * P, :])
        for mb in range(NQ // 512):
            xts = [xt_pool.tile([P, 512], FPR, name=f"xtv{rep}_{i}", tag="xt")
                   for i in range(KD)]
            for d in range(KD):
                nc.gpsimd.dma_start(
                    xts[d][:], XT[d * P : (d + 1) * P, mb * 512 : (mb + 1) * 512]
                )
            for m2 in range(4):
                m = mb * 4 + m2
                for eh in range(2):
                    ps = mm_ps.tile([P, 512], FP, name="mm", tag="mm")
                    for d in range(KD):
                        MM(ps[:], xts[d][:, m2 * P : (m2 + 1) * P],
                           wv[d][:, eh * 512 : (eh + 1) * 512],
                           start=(d == 0), stop=False)
                    MM(ps[:], ones_row[:], bv_row[:, eh * 512 : (eh + 1) * 512],
                       start=False, stop=True)
                    st = kst_pool.tile([P, 512], FPR, name="vst", tag="kst")
                    nc.vector.tensor_copy(st[:], ps[:])
                    nc.gpsimd.dma_start(
                        vown[m * P : (m + 1) * P, eh * 512 : (eh + 1) * 512], st[:]
                    )
        nc.gpsimd.collective_compute(
            "AllGather", mybir.AluOpType.bypass, replica_groups=PAIRS,
            ins=[vown[:]], outs=[vfull[:]],
        )
        for m in range(MC):
            nc.gpsimd.dma_start(vt[m][:], vfull[m * P : (m + 1) * P, :])

    # ---------------- attention ----------------
    with (
        tc.tile_pool(name=f"kts{rep}", bufs=16) as kts_pool,
        tc.tile_pool(name=f"pt{rep}", bufs=MC + 1) as pt_pool,
        tc.tile_pool(name=f"ostage{rep}", bufs=2) as ostage,
        tc.tile_pool(name=f"rec{rep}", bufs=4) as rec_pool,
    ):
        for nh in range(2):
            pts = []
            for mb in range(MB):
                kts = [kts_pool.tile([P, 512], FPR, name=f"kts{rep}_{i}", tag="kts")
                       for i in range(KE)]
                half, lb = mb // 2, mb % 2
                for e in range(KE):
                    nc.gpsimd.dma_start(
                        kts[e][:],
                        ktfull[half * D + e * P : half * D + (e + 1) * P,
                               lb * 512 : (lb + 1) * 512],
                    )
                for m2 in range(4):
                    st = st_ps.tile([P, 512], FP, name="st", tag="st")
                    for e in range(KE):
                        MM(st[:], kts[e][:, m2 * P : (m2 + 1) * P],
                           qt[e][:, nh * 512 : (nh + 1) * 512],
                           start=(e == 0), stop=(e == KE - 1))
                    ptm = pt_pool.tile([P, 512], FPR, name="pt", tag="pt")
                    nc.scalar.activation(
                        ptm[:], st[:], mybir.ActivationFunctionType.Exp
                    )
                    pts.append(ptm)
            for ns in range(4):
                o0 = out_ps.tile([P, 512], FP, name="o0", tag="out")
                o1 = out_ps.tile([P, 512], FP, name="o1", tag="out")
                den = mm_ps.tile([P, 512], FP, name="den", tag="mm")
                for m in range(MC):
                    lh = pts[m][:, ns * P : (ns + 1) * P]
                    MM(o0[:], lh, vt[m][:, 0:512],
                       start=(m == 0), stop=(m == MC - 1))
                    MM(o1[:], lh, vt[m][:, 512:1024],
                       start=(m == 0), stop=(m == MC - 1))
                    MM(den[:, 0:2], lh, ones_col[:],
                       start=(m == 0), stop=(m == MC - 1))
                rec = rec_pool.tile([P, 1], FP, name="rec", tag="rec")
                nc.vector.reciprocal(rec[:], den[:, 0:1])
                ost = ostage.tile([P, D], FP, name="ost", tag="ost")
                nc.vector.tensor_scalar_mul(ost[:, 0:512], o0[:], rec[:])
                nc.vector.tensor_scalar_mul(ost[:, 512:1024], o1[:], rec[:])
                nrow = nh * 512 + ns * P
                nc.gpsimd.dma_start(OUT[nrow : nrow + P, :], ost[:])


def build_bass(split=True, reps=1):
    nc = bass.Bass()
    XT = nc.declare_dram_parameter("XT", [D, NQ], FP, isOutput=False)
    Wq = nc.declare_dram_parameter("Wq", [D, D], FP, isOutput=False)
    Wk = nc.declare_dram_parameter("Wk", [D, D], FP, isOutput=False)
    Wv = nc.declare_dram_parameter("Wv", [D, D], FP, isOutput=False)
    BQ = nc.declare_dram_parameter("bq_t", [P, KE], FP, isOutput=False)
    BK = nc.declare_dram_parameter("bk_t", [P, KE], FP, isOutput=False)
    BV = nc.declare_dram_parameter("bv_row", [1, D], FP, isOutput=False)
    ONESC = nc.declare_dram_parameter("ones_col", [P, 2], FP, isOutput=False)
    ONESR = nc.declare_dram_parameter("ones_row", [1, P], FP, isOutput=False)
    OUT = nc.declare_dram_parameter("OUT", [NQ, D], FP, isOutput=True)

    with tile.TileContext(nc) as tc:
        with (
            tc.tile_pool(name="misc", bufs=1) as misc,
            tc.tile_pool(name="vt", bufs=MC) as v_pool,
            tc.tile_pool(name="qt", bufs=KE) as qt_pool,
            tc.tile_pool(name="mmps", bufs=2, space="PSUM") as mm_ps,
            tc.tile_pool(name="stps", bufs=2, space="PSUM") as st_ps,
            tc.tile_pool(name="outps", bufs=4, space="PSUM") as out_ps,
        ):
            bq_t = misc.tile([P, KE], FP, tag="bq")
            bk_t = misc.tile([P, KE], FP, tag="bk")
            bv_row = misc.tile([1, D], FPR, tag="bv")
            ones_col = misc.tile([P, 2], FPR, tag="onc")
            ones_row = misc.tile([1, P], FPR, tag="onr")
            nc.gpsimd.dma_start(bq_t[:], BQ[:])
            nc.gpsimd.dma_start(bk_t[:], BK[:])
            nc.gpsimd.dma_start(bv_row[:], BV[:])
            nc.gpsimd.dma_start(ones_col[:], ONESC[:])
            nc.gpsimd.dma_start(ones_row[:], ONESR[:])

            params = (XT, Wq, Wk, Wv, OUT)
            consts = (bq_t, bk_t, bv_row, ones_col, ones_row)
            pools = (v_pool, qt_pool, mm_ps, st_ps, out_ps)
            for rep in range(reps):
                _emit_body(nc, tc, rep, params, consts, pools)

    if split:
        _split_sync_waits(nc)
    return nc


_CACHE = {}


def _get_runner(reps=1, donate=True):
    """Compile once; return fn(in_maps) -> list[dict] running SPMD on 8 cores.

    reps>1 repeats the whole kernel body inside the NEFF (used for timing:
    slope over reps isolates per-body device time from dispatch overhead).
    """
    key = (reps, donate)
    if key in _CACHE:
        return _CACHE[key]

    import jax
    from jax.experimental.shard_map import shard_map
    from jax.sharding import Mesh, PartitionSpec

    from concourse import bass2jax

    nc = build_bass(reps=reps)
    bass2jax.install_neuronx_cc_hook()

    partition_name = (
        nc.partition_id_tensor.name if nc.partition_id_tensor else None
    )
    in_names, out_names, out_avals, zero_outs = [], [], [], []
    for alloc in nc.m.functions[0].allocations:
        if not isinstance(alloc, mybir.MemoryLocationSet):
            continue
        name = alloc.memorylocations[0].name
        if alloc.kind == "ExternalInput":
            if name != partition_name:
                in_names.append(name)
        elif alloc.kind == "ExternalOutput":
            shape = tuple(alloc.tensor_shape)
            dtype = mybir.dt.np(alloc.dtype)
            out_names.append(name)
            out_avals.append(jax.core.ShapedArray(shape, dtype))
            zero_outs.append(np.zeros(shape, dtype))
    n_params = len(in_names)
    n_outs = len(out_avals)
    all_in_names = list(in_names) + list(out_names)
    if partition_name is not None:
        all_in_names.append(partition_name)
    donate_idx = tuple(range(n_params, n_params + n_outs))

    def _body(*args):
        operands = list(args)
        if partition_name is not None:
            operands.append(bass2jax.partition_id_tensor())
        outs = bass2jax._bass_exec_p.bind(
            *operands,
            out_avals=tuple(out_avals),
            in_names=tuple(all_in_names),
            out_names=tuple(out_names),
            lowering_input_output_aliases=(),
            sim_require_finite=True,
            sim_require_nnan=True,
            nc=nc,
        )
        return tuple(outs)

    devices = jax.devices()[:NCORES]
    mesh = Mesh(np.asarray(devices), ("core",))
    in_specs = (PartitionSpec("core"),) * (n_params + n_outs)
    out_specs = (PartitionSpec("core"),) * n_outs
    sharded = jax.jit(
        shard_map(
            _body, mesh=mesh, in_specs=in_specs, out_specs=out_specs,
            check_rep=False,
        ),
        donate_argnums=donate_idx if donate else (),
        keep_unused=True,
    )

    def run(in_maps):
        import jax as _jax

        per_core = [[np.asarray(m[name]) for name in in_names] for m in in_maps]
        concat_in = [
            np.concatenate([per_core[c][i] for c in range(NCORES)], axis=0)
            for i in range(n_params)
        ]
        concat_zero = [np.concatenate([z] * NCORES, axis=0) for z in zero_outs]
        outs = sharded(*concat_in, *concat_zero)
        outs = [np.asarray(o) for o in _jax.block_until_ready(outs)]
        results = []
        for c in range(NCORES):
            r = {}
            for i, name in enumerate(out_names):
                d0 = out_avals[i].shape[0]
                r[name] = outs[i][c * d0 : (c + 1) * d0]
            results.append(r)
        return results

    run.sharded = sharded
    run.n_params = n_params
    run.in_names = in_names
    run.zero_outs = zero_outs
    _CACHE[key] = run
    return run


def _in_maps(X, Wq, bq, Wk, bk, Wv, bv):
    X = np.asarray(X, np.float32)
    maps = []
    bq_t = np.ascontiguousarray(np.asarray(bq, np.float32).reshape(KE, P).T)
    bk_t = np.ascontiguousarray(np.asarray(bk, np.float32).reshape(KE, P).T)
    bv_row = np.ascontiguousarray(np.asarray(bv, np.float32).reshape(1, D))
    Wq = np.ascontiguousarray(np.asarray(Wq, np.float32))
    Wk = np.ascontiguousarray(np.asarray(Wk, np.float32))
    Wv = np.ascontiguousarray(np.asarray(Wv, np.float32))
    for c in range(NCORES):
        b, h = c // 2, c % 2
        XT = np.ascontiguousarray(X[b, h * NQ : (h + 1) * NQ].T)
        maps.append(
            dict(XT=XT, Wq=Wq, Wk=Wk, Wv=Wv, bq_t=bq_t, bk_t=bk_t,
                 bv_row=bv_row, ones_col=np.ones((P, 2), np.float32),
                 ones_row=np.ones((1, P), np.float32))
        )
    return maps


def kernel(X, Wq, bq, Wk, bk, Wv, bv):
    run = _get_runner()
    results = run(_in_maps(X, Wq, bq, Wk, bk, Wv, bv))
    out = np.empty((B, N, D), np.float32)
    for c in range(NCORES):
        b, h = c // 2, c % 2
        out[b, h * NQ : (h + 1) * NQ, :] = results[c]["OUT"]
    return out


# revision 18
# speedup vs baseline: 1.2728x; 1.2728x over previous
"""Trainium2 Bass kernel for a single-head unscaled-softmax attention layer.

Reference computation (fp32):
    q = X @ Wq + bq ; k = X @ Wk + bk ; v = X @ Wv + bv        X: [B=4, N=2048, D=1024]
    out = softmax(q @ k^T, axis=-1) @ v                         (no 1/sqrt(d) scale)

Sharding: 8 cores = (batch b, sequence half h). Each core computes attention
for its 1024 query rows against the full 2048 keys of its batch (K/V
projections are recomputed per core pair - flash-style seq-block parallel,
as the single-head softmax couples the full feature dim). Attention is
permutation-invariant over keys, so each core receives X[b]^T with its own
query rows ordered first; the kernel is identical SPMD across all 8 cores.

Per-core kernel (matmuls in float32r - fp32 storage, fast PE mode):
  phase K: K^T[e,m] = Wk^T X^T   -> spilled to DRAM (streamed back later)
  phase Q: Q^T[e,n] = Wq^T X^T   -> SBUF resident
  phase V: V[m,e]   = X Wv       -> SBUF resident (bias via rank-1 matmul)
  attention (2 halves of 512 query rows):
    S^T[m,n] = K Q^T      (psum, contract e over K^T/Q^T chunks)
    P^T      = exp(S^T)   (ACT, no max subtraction - logits < ~60 so exp
                           stays in fp32 range; softmax is shift-invariant)
    out[n,e] = P^T.T @ V  (contract m over all 16 m-chunks in psum)
    den[n]   = P^T.T @ 1  (rides the same stationary weights)
    out     /= den        (DVE reciprocal + per-partition scale)
"""

import numpy as np

import concourse.bass as bass
import concourse.mybir as mybir
import concourse.tile as tile

B, N, D = 4, 2048, 1024
NCORES = 8
P = 128
NQ = N // 2          # query rows per core
KD = D // P          # 8 contraction chunks over d_in
KE = D // P          # 8 chunks over d_out (e)
MC = N // P          # 16 key chunks of 128
MB = N // 512        # 4 key blocks of 512
FP = mybir.dt.float32
FPR = mybir.dt.float32r


def _split_sync_waits(nc, max_waits=1):
    """Walrus codegen on this container accepts at most one sync-wait command
    per instruction; hoist excess waits onto NoOps injected just before the
    instruction on the same engine (engines execute in order, so blocking at
    the NoOp is equivalent)."""
    mb = mybir
    for fn in nc.m.functions:
        for bb in fn.blocks:
            insts = list(bb.instructions)
            new = []
            changed = False
            for inst in insts:
                si = getattr(inst, "sync_info", None)
                if si is not None and si.on_wait and len(si.on_wait) > max_waits:
                    waits = list(si.on_wait)
                    keep = waits[-max_waits:]
                    excess = waits[:-max_waits]
                    for i in range(0, len(excess), max_waits):
                        chunk = excess[i : i + max_waits]
                        nop = mb.InstNoOp(
                            name=f"{inst.name}-sw{i}", ins=[], outs=[],
                            engine=inst.engine,
                        )
                        nop.sync_info = mb.SyncInfo(on_wait=chunk, on_update=[])
                        new.append(nop)
                    inst.sync_info = mb.SyncInfo(
                        on_wait=keep, on_update=list(si.on_update or [])
                    )
                    changed = True
                new.append(inst)
            if changed:
                bb.instructions = new


def _emit_body(nc, tc, rep, params, consts, pools):
    """One full attention computation for this core's shard."""
    XT, Wq, Wk, Wv, OUT = params
    bq_t, bk_t, bv_row, ones_col, ones_row = consts
    v_pool, qt_pool, ktdram, mm_ps, st_ps, out_ps = pools
    MM = nc.tensor.matmul

    KT = ktdram.tile([D, N], FPR, name=f"KT{rep}", tag="KT")
    vt = [v_pool.tile([P, D], FPR, name=f"vt{rep}_{i}", tag="vt") for i in range(MC)]
    qt = [qt_pool.tile([P, NQ], FPR, name=f"qt{rep}_{i}", tag="qt") for i in range(KE)]

    # ---------------- projections ----------------
    with (
        tc.tile_pool(name=f"w{rep}", bufs=12) as w_pool,
        tc.tile_pool(name=f"xt{rep}", bufs=22) as xt_pool,
        tc.tile_pool(name=f"kst{rep}", bufs=6) as kst_pool,
    ):
        # K phase: K^T[e, m] -> DRAM
        wk = [w_pool.tile([P, D], FPR, name=f"wk{rep}_{i}", tag="w") for i in range(KD)]
        for d in range(KD):
            nc.sync.dma_start(wk[d][:], Wk[d * P : (d + 1) * P, :])
        for mb in range(MB):
            xts = [xt_pool.tile([P, 512], FPR, name=f"xtk{rep}_{i}", tag="xt")
                   for i in range(KD)]
            for d in range(KD):
                nc.sync.dma_start(
                    xts[d][:], XT[d * P : (d + 1) * P, mb * 512 : (mb + 1) * 512]
                )
            for e in range(KE):
                ps = mm_ps.tile([P, 512], FP, name="mm", tag="mm")
                for d in range(KD):
                    MM(ps[:], wk[d][:, e * P : (e + 1) * P], xts[d][:],
                       start=(d == 0), stop=(d == KD - 1))
                st = kst_pool.tile([P, 512], FPR, name="kst", tag="kst")
                nc.vector.tensor_scalar_add(st[:], ps[:], bk_t[:, e : e + 1])
                nc.scalar.dma_start(
                    KT[e * P : (e + 1) * P, mb * 512 : (mb + 1) * 512], st[:]
                )

        # Q phase: Q^T[e, n] -> SBUF (query rows are XT columns 0..NQ)
        wq = [w_pool.tile([P, D], FPR, name=f"wq{rep}_{i}", tag="w") for i in range(KD)]
        for d in range(KD):
            nc.sync.dma_start(wq[d][:], Wq[d * P : (d + 1) * P, :])
        for mb in range(NQ // 512):
            xts = [xt_pool.tile([P, 512], FPR, name=f"xtq{rep}_{i}", tag="xt")
                   for i in range(KD)]
            for d in range(KD):
                nc.sync.dma_start(
                    xts[d][:], XT[d * P : (d + 1) * P, mb * 512 : (mb + 1) * 512]
                )
            for e in range(KE):
                ps = mm_ps.tile([P, 512], FP, name="mm", tag="mm")
                for d in range(KD):
                    MM(ps[:], wq[d][:, e * P : (e + 1) * P], xts[d][:],
                       start=(d == 0), stop=(d == KD - 1))
                nc.vector.tensor_scalar_add(
                    qt[e][:, mb * 512 : (mb + 1) * 512], ps[:], bq_t[:, e : e + 1]
                )

        # V phase: V[m, e] -> SBUF
        wv = [w_pool.tile([P, D], FPR, name=f"wv{rep}_{i}", tag="w") for i in range(KD)]
        for d in range(KD):
            nc.sync.dma_start(wv[d][:], Wv[d * P : (d + 1) * P, :])
        for mb in range(MB):
            xts = [xt_pool.tile([P, 512], FPR, name=f"xtv{rep}_{i}", tag="xt")
                   for i in range(KD)]
            for d in range(KD):
                nc.sync.dma_start(
                    xts[d][:], XT[d * P : (d + 1) * P, mb * 512 : (mb + 1) * 512]
                )
            for m2 in range(4):
                m = mb * 4 + m2
                for eh in range(2):
                    ps = mm_ps.tile([P, 512], FP, name="mm", tag="mm")
                    for d in range(KD):
                        MM(ps[:], xts[d][:, m2 * P : (m2 + 1) * P],
                           wv[d][:, eh * 512 : (eh + 1) * 512],
                           start=(d == 0), stop=False)
                    MM(ps[:], ones_row[:], bv_row[:, eh * 512 : (eh + 1) * 512],
                       start=False, stop=True)
                    nc.vector.tensor_copy(vt[m][:, eh * 512 : (eh + 1) * 512], ps[:])

    # ---------------- attention ----------------
    with (
        tc.tile_pool(name=f"kts{rep}", bufs=3) as kts_pool,
        tc.tile_pool(name=f"pt{rep}", bufs=MC + 1) as pt_pool,
        tc.tile_pool(name=f"ostage{rep}", bufs=4) as ostage,
        tc.tile_pool(name=f"rec{rep}", bufs=4) as rec_pool,
    ):
        for nh in range(2):
            pts = []
            for mb in range(MB):
                ktsb = kts_pool.tile([P, KE, 512], FPR, name=f"kts{rep}", tag="kts")
                nc.scalar.dma_start(
                    ktsb[:],
                    KT[:, mb * 512 : (mb + 1) * 512].rearrange(
                        "(e p) m -> p e m", p=P
                    ),
                )
                kts = [ktsb[:, i, :] for i in range(KE)]
                for m2 in range(4):
                    st = st_ps.tile([P, 512], FP, name="st", tag="st")
                    for e in range(KE):
                        MM(st[:], kts[e][:, m2 * P : (m2 + 1) * P],
                           qt[e][:, nh * 512 : (nh + 1) * 512],
                           start=(e == 0), stop=(e == KE - 1))
                    ptm = pt_pool.tile([P, 512], FPR, name="pt", tag="pt")
                    nc.scalar.activation(
                        ptm[:], st[:], mybir.ActivationFunctionType.Exp
                    )
                    pts.append(ptm)
            for ns in range(4):
                o0 = out_ps.tile([P, 512], FP, name="o0", tag="out")
                o1 = out_ps.tile([P, 512], FP, name="o1", tag="out")
                den = mm_ps.tile([P, 512], FP, name="den", tag="mm")
                for m in range(MC):
                    lh = pts[m][:, ns * P : (ns + 1) * P]
                    MM(o0[:], lh, vt[m][:, 0:512],
                       start=(m == 0), stop=(m == MC - 1))
                    MM(o1[:], lh, vt[m][:, 512:1024],
                       start=(m == 0), stop=(m == MC - 1))
                    MM(den[:, 0:2], lh, ones_col[:],
                       start=(m == 0), stop=(m == MC - 1))
                rec = rec_pool.tile([P, 1], FP, name="rec", tag="rec")
                nc.vector.reciprocal(rec[:], den[:, 0:1])
                ost = ostage.tile([P, D], FP, name="ost", tag="ost")
                nc.vector.tensor_scalar_mul(ost[:, 0:512], o0[:], rec[:])
                nc.vector.tensor_scalar_mul(ost[:, 512:1024], o1[:], rec[:])
                nrow = nh * 512 + ns * P
                nc.scalar.dma_start(OUT[nrow : nrow + P, :], ost[:])


def build_bass(split=True, reps=1):
    nc = bass.Bass()
    XT = nc.declare_dram_parameter("XT", [D, N], FPR, isOutput=False)
    Wq = nc.declare_dram_parameter("Wq", [D, D], FPR, isOutput=False)
    Wk = nc.declare_dram_parameter("Wk", [D, D], FPR, isOutput=False)
    Wv = nc.declare_dram_parameter("Wv", [D, D], FPR, isOutput=False)
    BQ = nc.declare_dram_parameter("bq_t", [P, KE], FP, isOutput=False)
    BK = nc.declare_dram_parameter("bk_t", [P, KE], FP, isOutput=False)
    BV = nc.declare_dram_parameter("bv_row", [1, D], FPR, isOutput=False)
    ONESC = nc.declare_dram_parameter("ones_col", [P, 2], FPR, isOutput=False)
    ONESR = nc.declare_dram_parameter("ones_row", [1, P], FPR, isOutput=False)
    OUT = nc.declare_dram_parameter("OUT", [NQ, D], FP, isOutput=True)

    with tile.TileContext(nc) as tc:
        with (
            tc.tile_pool(name="misc", bufs=1) as misc,
            tc.tile_pool(name="vt", bufs=MC) as v_pool,
            tc.tile_pool(name="qt", bufs=KE) as qt_pool,
            tc.tile_pool(name="ktdram", bufs=1, space="DRAM") as ktdram,
            tc.tile_pool(name="mmps", bufs=2, space="PSUM") as mm_ps,
            tc.tile_pool(name="stps", bufs=2, space="PSUM") as st_ps,
            tc.tile_pool(name="outps", bufs=4, space="PSUM") as out_ps,
        ):
            bq_t = misc.tile([P, KE], FP, tag="bq")
            bk_t = misc.tile([P, KE], FP, tag="bk")
            bv_row = misc.tile([1, D], FPR, tag="bv")
            ones_col = misc.tile([P, 2], FPR, tag="onc")
            ones_row = misc.tile([1, P], FPR, tag="onr")
            nc.sync.dma_start(bq_t[:], BQ[:])
            nc.sync.dma_start(bk_t[:], BK[:])
            nc.sync.dma_start(bv_row[:], BV[:])
            nc.sync.dma_start(ones_col[:], ONESC[:])
            nc.sync.dma_start(ones_row[:], ONESR[:])

            params = (XT, Wq, Wk, Wv, OUT)
            consts = (bq_t, bk_t, bv_row, ones_col, ones_row)
            pools = (v_pool, qt_pool, ktdram, mm_ps, st_ps, out_ps)
            for rep in range(reps):
                _emit_body(nc, tc, rep, params, consts, pools)

    if split:
        _split_sync_waits(nc)
    return nc


_CACHE = {}


def _get_runner(reps=1, donate=True):
    """Compile once; return fn(in_maps) -> list[dict] running SPMD on 8 cores.

    reps>1 repeats the whole kernel body inside the NEFF (used for timing:
    slope over reps isolates per-body device time from dispatch overhead).
    """
    key = (reps, donate)
    if key in _CACHE:
        return _CACHE[key]

    import jax
    from jax.experimental.shard_map import shard_map
    from jax.sharding import Mesh, PartitionSpec

    from concourse import bass2jax

    nc = build_bass(reps=reps)
    bass2jax.install_neuronx_cc_hook()

    partition_name = (
        nc.partition_id_tensor.name if nc.partition_id_tensor else None
    )
    in_names, out_names, out_avals, zero_outs = [], [], [], []
    for alloc in nc.m.functions[0].allocations:
        if not isinstance(alloc, mybir.MemoryLocationSet):
            continue
        name = alloc.memorylocations[0].name
        if alloc.kind == "ExternalInput":
            if name != partition_name:
                in_names.append(name)
        elif alloc.kind == "ExternalOutput":
            shape = tuple(alloc.tensor_shape)
            dtype = mybir.dt.np(alloc.dtype)
            out_names.append(name)
            out_avals.append(jax.core.ShapedArray(shape, dtype))
            zero_outs.append(np.zeros(shape, dtype))
    n_params = len(in_names)
    n_outs = len(out_avals)
    all_in_names = list(in_names) + list(out_names)
    if partition_name is not None:
        all_in_names.append(partition_name)
    donate_idx = tuple(range(n_params, n_params + n_outs))

    def _body(*args):
        operands = list(args)
        if partition_name is not None:
            operands.append(bass2jax.partition_id_tensor())
        outs = bass2jax._bass_exec_p.bind(
            *operands,
            out_avals=tuple(out_avals),
            in_names=tuple(all_in_names),
            out_names=tuple(out_names),
            lowering_input_output_aliases=(),
            sim_require_finite=True,
            sim_require_nnan=True,
            nc=nc,
        )
        return tuple(outs)

    devices = jax.devices()[:NCORES]
    mesh = Mesh(np.asarray(devices), ("core",))
    in_specs = (PartitionSpec("core"),) * (n_params + n_outs)
    out_specs = (PartitionSpec("core"),) * n_outs
    sharded = jax.jit(
        shard_map(
            _body, mesh=mesh, in_specs=in_specs, out_specs=out_specs,
            check_rep=False,
        ),
        donate_argnums=donate_idx if donate else (),
        keep_unused=True,
    )

    def run(in_maps):
        import jax as _jax

        per_core = [[np.asarray(m[name]) for name in in_names] for m in in_maps]
        concat_in = [
            np.concatenate([per_core[c][i] for c in range(NCORES)], axis=0)
            for i in range(n_params)
        ]
        concat_zero = [np.concatenate([z] * NCORES, axis=0) for z in zero_outs]
        outs = sharded(*concat_in, *concat_zero)
        outs = [np.asarray(o) for o in _jax.block_until_ready(outs)]
        results = []
        for c in range(NCORES):
            r = {}
            for i, name in enumerate(out_names):
                d0 = out_avals[i].shape[0]
                r[name] = outs[i][c * d0 : (c + 1) * d0]
            results.append(r)
        return results

    run.sharded = sharded
    run.n_params = n_params
    run.in_names = in_names
    run.zero_outs = zero_outs
    _CACHE[key] = run
    return run


def _in_maps(X, Wq, bq, Wk, bk, Wv, bv):
    X = np.asarray(X, np.float32)
    maps = []
    bq_t = np.ascontiguousarray(np.asarray(bq, np.float32).reshape(KE, P).T)
    bk_t = np.ascontiguousarray(np.asarray(bk, np.float32).reshape(KE, P).T)
    bv_row = np.ascontiguousarray(np.asarray(bv, np.float32).reshape(1, D))
    Wq = np.ascontiguousarray(np.asarray(Wq, np.float32))
    Wk = np.ascontiguousarray(np.asarray(Wk, np.float32))
    Wv = np.ascontiguousarray(np.asarray(Wv, np.float32))
    for c in range(NCORES):
        b, h = c // 2, c % 2
        Xb = X[b]
        rows = np.concatenate(
            [Xb[h * NQ : (h + 1) * NQ], Xb[(1 - h) * NQ : (2 - h) * NQ]], axis=0
        )
        XT = np.ascontiguousarray(rows.T)
        maps.append(
            dict(XT=XT, Wq=Wq, Wk=Wk, Wv=Wv, bq_t=bq_t, bk_t=bk_t,
                 bv_row=bv_row, ones_col=np.ones((P, 2), np.float32),
                 ones_row=np.ones((1, P), np.float32))
        )
    return maps


def kernel(X, Wq, bq, Wk, bk, Wv, bv):
    run = _get_runner()
    results = run(_in_maps(X, Wq, bq, Wk, bk, Wv, bv))
    out = np.empty((B, N, D), np.float32)
    for c in range(NCORES):
        b, h = c // 2, c % 2
        out[b, h * NQ : (h + 1) * NQ, :] = results[c]["OUT"]
    return out


# revision 19
# speedup vs baseline: 1.3226x; 1.0392x over previous
"""Trainium2 Bass kernel for a single-head unscaled-softmax attention layer.

Reference computation (fp32):
    q = X @ Wq + bq ; k = X @ Wk + bk ; v = X @ Wv + bv        X: [B=4, N=2048, D=1024]
    out = softmax(q @ k^T, axis=-1) @ v                         (no 1/sqrt(d) scale)

Sharding: 8 cores = (batch b, sequence half h). Each core computes attention
for its 1024 query rows against the full 2048 keys of its batch (K/V
projections are recomputed per core pair - flash-style seq-block parallel,
as the single-head softmax couples the full feature dim). Attention is
permutation-invariant over keys, so each core receives X[b]^T with its own
query rows ordered first; the kernel is identical SPMD across all 8 cores.

Per-core kernel (matmuls in float32r - fp32 storage, fast PE mode):
  phase K: K^T[e,m] = Wk^T X^T   -> spilled to DRAM (streamed back later)
  phase Q: Q^T[e,n] = Wq^T X^T   -> SBUF resident
  phase V: V[m,e]   = X Wv       -> SBUF resident (bias via rank-1 matmul)
  attention (2 halves of 512 query rows):
    S^T[m,n] = K Q^T      (psum, contract e over K^T/Q^T chunks)
    P^T      = exp(S^T)   (ACT, no max subtraction - logits < ~60 so exp
                           stays in fp32 range; softmax is shift-invariant)
    out[n,e] = P^T.T @ V  (contract m over all 16 m-chunks in psum)
    den[n]   = P^T.T @ 1  (rides the same stationary weights)
    out     /= den        (DVE reciprocal + per-partition scale)
"""

import numpy as np

import concourse.bass as bass
import concourse.mybir as mybir
import concourse.tile as tile

B, N, D = 4, 2048, 1024
NCORES = 8
P = 128
NQ = N // 2          # query rows per core
KD = D // P          # 8 contraction chunks over d_in
KE = D // P          # 8 chunks over d_out (e)
MC = N // P          # 16 key chunks of 128
MB = N // 512        # 4 key blocks of 512
FP = mybir.dt.float32
FPR = mybir.dt.float32r


def _split_sync_waits(nc, max_waits=1):
    """Walrus codegen on this container accepts at most one sync-wait command
    per instruction; hoist excess waits onto NoOps injected just before the
    instruction on the same engine (engines execute in order, so blocking at
    the NoOp is equivalent)."""
    mb = mybir
    for fn in nc.m.functions:
        for bb in fn.blocks:
            insts = list(bb.instructions)
            new = []
            changed = False
            for inst in insts:
                si = getattr(inst, "sync_info", None)
                if si is not None and si.on_wait and len(si.on_wait) > max_waits:
                    waits = list(si.on_wait)
                    keep = waits[-max_waits:]
                    excess = waits[:-max_waits]
                    for i in range(0, len(excess), max_waits):
                        chunk = excess[i : i + max_waits]
                        nop = mb.InstNoOp(
                            name=f"{inst.name}-sw{i}", ins=[], outs=[],
                            engine=inst.engine,
                        )
                        nop.sync_info = mb.SyncInfo(on_wait=chunk, on_update=[])
                        new.append(nop)
                    inst.sync_info = mb.SyncInfo(
                        on_wait=keep, on_update=list(si.on_update or [])
                    )
                    changed = True
                new.append(inst)
            if changed:
                bb.instructions = new


def _emit_body(nc, tc, rep, params, consts, pools):
    """One full attention computation for this core's shard."""
    XT, Wq, Wk, Wv, OUT = params
    bq_t, bk_t, bv_row, ones_col, ones_row = consts
    v_pool, qt_pool, ktdram, mm_ps, st_ps, out_ps = pools
    MM = nc.tensor.matmul

    KT = ktdram.tile([D, N], FPR, name=f"KT{rep}", tag="KT")
    vt = [v_pool.tile([P, D], FPR, name=f"vt{rep}_{i}", tag="vt") for i in range(MC)]
    qt = [qt_pool.tile([P, NQ], FPR, name=f"qt{rep}_{i}", tag="qt") for i in range(KE)]

    # ---------------- projections ----------------
    with (
        tc.tile_pool(name=f"w{rep}", bufs=12) as w_pool,
        tc.tile_pool(name=f"xt{rep}", bufs=22) as xt_pool,
        tc.tile_pool(name=f"kst{rep}", bufs=6) as kst_pool,
    ):
        # K phase: K^T[e, m] -> DRAM
        wk = [w_pool.tile([P, D], FPR, name=f"wk{rep}_{i}", tag="w") for i in range(KD)]
        for d in range(KD):
            nc.sync.dma_start(wk[d][:], Wk[d * P : (d + 1) * P, :])
        for mb in range(MB):
            xts = [xt_pool.tile([P, 512], FPR, name=f"xtk{rep}_{i}", tag="xt")
                   for i in range(KD)]
            for d in range(KD):
                nc.sync.dma_start(
                    xts[d][:], XT[d * P : (d + 1) * P, mb * 512 : (mb + 1) * 512]
                )
            for e in range(KE):
                ps = mm_ps.tile([P, 512], FP, name="mm", tag="mm")
                for d in range(KD):
                    MM(ps[:], wk[d][:, e * P : (e + 1) * P], xts[d][:],
                       start=(d == 0), stop=(d == KD - 1))
                st = kst_pool.tile([P, 512], FPR, name="kst", tag="kst")
                nc.vector.tensor_scalar_add(st[:], ps[:], bk_t[:, e : e + 1])
                nc.scalar.dma_start(
                    KT[e * P : (e + 1) * P, mb * 512 : (mb + 1) * 512], st[:]
                )

        # Q phase: Q^T[e, n] -> SBUF (query rows are XT columns 0..NQ)
        wq = [w_pool.tile([P, D], FPR, name=f"wq{rep}_{i}", tag="w") for i in range(KD)]
        for d in range(KD):
            nc.sync.dma_start(wq[d][:], Wq[d * P : (d + 1) * P, :])
        for mb in range(NQ // 512):
            xts = [xt_pool.tile([P, 512], FPR, name=f"xtq{rep}_{i}", tag="xt")
                   for i in range(KD)]
            for d in range(KD):
                nc.sync.dma_start(
                    xts[d][:], XT[d * P : (d + 1) * P, mb * 512 : (mb + 1) * 512]
                )
            for e in range(KE):
                ps = mm_ps.tile([P, 512], FP, name="mm", tag="mm")
                for d in range(KD):
                    MM(ps[:], wq[d][:, e * P : (e + 1) * P], xts[d][:],
                       start=(d == 0), stop=(d == KD - 1))
                nc.vector.tensor_scalar_add(
                    qt[e][:, mb * 512 : (mb + 1) * 512], ps[:], bq_t[:, e : e + 1]
                )

        # V phase: V[m, e] -> SBUF
        wv = [w_pool.tile([P, D], FPR, name=f"wv{rep}_{i}", tag="w") for i in range(KD)]
        for d in range(KD):
            nc.sync.dma_start(wv[d][:], Wv[d * P : (d + 1) * P, :])
        for mb in range(MB):
            xts = [xt_pool.tile([P, 512], FPR, name=f"xtv{rep}_{i}", tag="xt")
                   for i in range(KD)]
            for d in range(KD):
                nc.sync.dma_start(
                    xts[d][:], XT[d * P : (d + 1) * P, mb * 512 : (mb + 1) * 512]
                )
            for m2 in range(4):
                m = mb * 4 + m2
                for eh in range(2):
                    ps = mm_ps.tile([P, 512], FP, name="mm", tag="mm")
                    for d in range(KD):
                        MM(ps[:], xts[d][:, m2 * P : (m2 + 1) * P],
                           wv[d][:, eh * 512 : (eh + 1) * 512],
                           start=(d == 0), stop=False)
                    MM(ps[:], ones_row[:], bv_row[:, eh * 512 : (eh + 1) * 512],
                       start=False, stop=True)
                    nc.vector.tensor_copy(vt[m][:, eh * 512 : (eh + 1) * 512], ps[:])

    # ---------------- attention ----------------
    with (
        tc.tile_pool(name=f"kts{rep}", bufs=3) as kts_pool,
        tc.tile_pool(name=f"pt{rep}", bufs=MC + 4) as pt_pool,
        tc.tile_pool(name=f"ostage{rep}", bufs=4) as ostage,
        tc.tile_pool(name=f"rec{rep}", bufs=4) as rec_pool,
    ):
        for nh in range(2):
            pts = []
            for mb in range(MB):
                ktsb = kts_pool.tile([P, KE, 512], FPR, name=f"kts{rep}", tag="kts")
                nc.scalar.dma_start(
                    ktsb[:],
                    KT[:, mb * 512 : (mb + 1) * 512].rearrange(
                        "(e p) m -> p e m", p=P
                    ),
                )
                kts = [ktsb[:, i, :] for i in range(KE)]
                for m2 in range(4):
                    st = st_ps.tile([P, 512], FP, name="st", tag="st")
                    for e in range(KE):
                        MM(st[:], kts[e][:, m2 * P : (m2 + 1) * P],
                           qt[e][:, nh * 512 : (nh + 1) * 512],
                           start=(e == 0), stop=(e == KE - 1))
                    ptm = pt_pool.tile([P, 512], FPR, name="pt", tag="pt")
                    nc.scalar.activation(
                        ptm[:], st[:], mybir.ActivationFunctionType.Exp
                    )
                    pts.append(ptm)
            for ns in range(4):
                o0 = out_ps.tile([P, 512], FP, name="o0", tag="out")
                o1 = out_ps.tile([P, 512], FP, name="o1", tag="out")
                den = mm_ps.tile([P, 512], FP, name="den", tag="mm")
                for m in range(MC):
                    lh = pts[m][:, ns * P : (ns + 1) * P]
                    MM(o0[:], lh, vt[m][:, 0:512],
                       start=(m == 0), stop=(m == MC - 1))
                    MM(o1[:], lh, vt[m][:, 512:1024],
                       start=(m == 0), stop=(m == MC - 1))
                    MM(den[:, 0:2], lh, ones_col[:],
                       start=(m == 0), stop=(m == MC - 1))
                rec = rec_pool.tile([P, 1], FP, name="rec", tag="rec")
                nc.vector.reciprocal(rec[:], den[:, 0:1])
                ost = ostage.tile([P, D], FP, name="ost", tag="ost")
                nc.vector.tensor_scalar_mul(ost[:, 0:512], o0[:], rec[:])
                nc.vector.tensor_scalar_mul(ost[:, 512:1024], o1[:], rec[:])
                nrow = nh * 512 + ns * P
                nc.scalar.dma_start(OUT[nrow : nrow + P, :], ost[:])


def build_bass(split=True, reps=1):
    nc = bass.Bass()
    XT = nc.declare_dram_parameter("XT", [D, N], FPR, isOutput=False)
    Wq = nc.declare_dram_parameter("Wq", [D, D], FPR, isOutput=False)
    Wk = nc.declare_dram_parameter("Wk", [D, D], FPR, isOutput=False)
    Wv = nc.declare_dram_parameter("Wv", [D, D], FPR, isOutput=False)
    BQ = nc.declare_dram_parameter("bq_t", [P, KE], FP, isOutput=False)
    BK = nc.declare_dram_parameter("bk_t", [P, KE], FP, isOutput=False)
    BV = nc.declare_dram_parameter("bv_row", [1, D], FPR, isOutput=False)
    ONESC = nc.declare_dram_parameter("ones_col", [P, 2], FPR, isOutput=False)
    ONESR = nc.declare_dram_parameter("ones_row", [1, P], FPR, isOutput=False)
    OUT = nc.declare_dram_parameter("OUT", [NQ, D], FP, isOutput=True)

    with tile.TileContext(nc) as tc:
        with (
            tc.tile_pool(name="misc", bufs=1) as misc,
            tc.tile_pool(name="vt", bufs=MC) as v_pool,
            tc.tile_pool(name="qt", bufs=KE) as qt_pool,
            tc.tile_pool(name="ktdram", bufs=1, space="DRAM") as ktdram,
            tc.tile_pool(name="mmps", bufs=2, space="PSUM") as mm_ps,
            tc.tile_pool(name="stps", bufs=2, space="PSUM") as st_ps,
            tc.tile_pool(name="outps", bufs=4, space="PSUM") as out_ps,
        ):
            bq_t = misc.tile([P, KE], FP, tag="bq")
            bk_t = misc.tile([P, KE], FP, tag="bk")
            bv_row = misc.tile([1, D], FPR, tag="bv")
            ones_col = misc.tile([P, 2], FPR, tag="onc")
            ones_row = misc.tile([1, P], FPR, tag="onr")
            nc.sync.dma_start(bq_t[:], BQ[:])
            nc.sync.dma_start(bk_t[:], BK[:])
            nc.sync.dma_start(bv_row[:], BV[:])
            nc.sync.dma_start(ones_col[:], ONESC[:])
            nc.sync.dma_start(ones_row[:], ONESR[:])

            params = (XT, Wq, Wk, Wv, OUT)
            consts = (bq_t, bk_t, bv_row, ones_col, ones_row)
            pools = (v_pool, qt_pool, ktdram, mm_ps, st_ps, out_ps)
            for rep in range(reps):
                _emit_body(nc, tc, rep, params, consts, pools)

    if split:
        _split_sync_waits(nc)
    return nc


_CACHE = {}


def _get_runner(reps=1, donate=True):
    """Compile once; return fn(in_maps) -> list[dict] running SPMD on 8 cores.

    reps>1 repeats the whole kernel body inside the NEFF (used for timing:
    slope over reps isolates per-body device time from dispatch overhead).
    """
    key = (reps, donate)
    if key in _CACHE:
        return _CACHE[key]

    import jax
    from jax.experimental.shard_map import shard_map
    from jax.sharding import Mesh, PartitionSpec

    from concourse import bass2jax

    nc = build_bass(reps=reps)
    bass2jax.install_neuronx_cc_hook()

    partition_name = (
        nc.partition_id_tensor.name if nc.partition_id_tensor else None
    )
    in_names, out_names, out_avals, zero_outs = [], [], [], []
    for alloc in nc.m.functions[0].allocations:
        if not isinstance(alloc, mybir.MemoryLocationSet):
            continue
        name = alloc.memorylocations[0].name
        if alloc.kind == "ExternalInput":
            if name != partition_name:
                in_names.append(name)
        elif alloc.kind == "ExternalOutput":
            shape = tuple(alloc.tensor_shape)
            dtype = mybir.dt.np(alloc.dtype)
            out_names.append(name)
            out_avals.append(jax.core.ShapedArray(shape, dtype))
            zero_outs.append(np.zeros(shape, dtype))
    n_params = len(in_names)
    n_outs = len(out_avals)
    all_in_names = list(in_names) + list(out_names)
    if partition_name is not None:
        all_in_names.append(partition_name)
    donate_idx = tuple(range(n_params, n_params + n_outs))

    def _body(*args):
        operands = list(args)
        if partition_name is not None:
            operands.append(bass2jax.partition_id_tensor())
        outs = bass2jax._bass_exec_p.bind(
            *operands,
            out_avals=tuple(out_avals),
            in_names=tuple(all_in_names),
            out_names=tuple(out_names),
            lowering_input_output_aliases=(),
            sim_require_finite=True,
            sim_require_nnan=True,
            nc=nc,
        )
        return tuple(outs)

    devices = jax.devices()[:NCORES]
    mesh = Mesh(np.asarray(devices), ("core",))
    in_specs = (PartitionSpec("core"),) * (n_params + n_outs)
    out_specs = (PartitionSpec("core"),) * n_outs
    sharded = jax.jit(
        shard_map(
            _body, mesh=mesh, in_specs=in_specs, out_specs=out_specs,
            check_rep=False,
        ),
        donate_argnums=donate_idx if donate else (),
        keep_unused=True,
    )

    def run(in_maps):
        import jax as _jax

        per_core = [[np.asarray(m[name]) for name in in_names] for m in in_maps]
        concat_in = [
            np.concatenate([per_core[c][i] for c in range(NCORES)], axis=0)
            for i in range(n_params)
        ]
        concat_zero = [np.concatenate([z] * NCORES, axis=0) for z in zero_outs]
        outs = sharded(*concat_in, *concat_zero)
        outs = [np.asarray(o) for o in _jax.block_until_ready(outs)]
        results = []
        for c in range(NCORES):
            r = {}
            for i, name in enumerate(out_names):
                d0 = out_avals[i].shape[0]
                r[name] = outs[i][c * d0 : (c + 1) * d0]
            results.append(r)
        return results

    run.sharded = sharded
    run.n_params = n_params
    run.in_names = in_names
    run.zero_outs = zero_outs
    _CACHE[key] = run
    return run


def _in_maps(X, Wq, bq, Wk, bk, Wv, bv):
    X = np.asarray(X, np.float32)
    maps = []
    bq_t = np.ascontiguousarray(np.asarray(bq, np.float32).reshape(KE, P).T)
    bk_t = np.ascontiguousarray(np.asarray(bk, np.float32).reshape(KE, P).T)
    bv_row = np.ascontiguousarray(np.asarray(bv, np.float32).reshape(1, D))
    Wq = np.ascontiguousarray(np.asarray(Wq, np.float32))
    Wk = np.ascontiguousarray(np.asarray(Wk, np.float32))
    Wv = np.ascontiguousarray(np.asarray(Wv, np.float32))
    for c in range(NCORES):
        b, h = c // 2, c % 2
        Xb = X[b]
        rows = np.concatenate(
            [Xb[h * NQ : (h + 1) * NQ], Xb[(1 - h) * NQ : (2 - h) * NQ]], axis=0
        )
        XT = np.ascontiguousarray(rows.T)
        maps.append(
            dict(XT=XT, Wq=Wq, Wk=Wk, Wv=Wv, bq_t=bq_t, bk_t=bk_t,
                 bv_row=bv_row, ones_col=np.ones((P, 2), np.float32),
                 ones_row=np.ones((1, P), np.float32))
        )
    return maps


def kernel(X, Wq, bq, Wk, bk, Wv, bv):
    run = _get_runner()
    results = run(_in_maps(X, Wq, bq, Wk, bk, Wv, bv))
    out = np.empty((B, N, D), np.float32)
    for c in range(NCORES):
        b, h = c // 2, c % 2
        out[b, h * NQ : (h + 1) * NQ, :] = results[c]["OUT"]
    return out


# revision 21
# speedup vs baseline: 1.3423x; 1.0148x over previous
"""Trainium2 Bass kernel for a single-head unscaled-softmax attention layer.

Reference computation (fp32):
    q = X @ Wq + bq ; k = X @ Wk + bk ; v = X @ Wv + bv        X: [B=4, N=2048, D=1024]
    out = softmax(q @ k^T, axis=-1) @ v                         (no 1/sqrt(d) scale)

Sharding: 8 cores = (batch b, sequence half h). Each core computes attention
for its 1024 query rows against the full 2048 keys of its batch (K/V
projections are recomputed per core pair - flash-style seq-block parallel,
as the single-head softmax couples the full feature dim). Attention is
permutation-invariant over keys, so each core receives X[b]^T with its own
query rows ordered first; the kernel is identical SPMD across all 8 cores.

Per-core kernel (matmuls in float32r - fp32 storage, fast PE mode):
  phase K: K^T[e,m] = Wk^T X^T   -> spilled to DRAM (streamed back later)
  phase Q: Q^T[e,n] = Wq^T X^T   -> SBUF resident
  phase V: V[m,e]   = X Wv       -> SBUF resident (bias via rank-1 matmul)
  attention (2 halves of 512 query rows):
    S^T[m,n] = K Q^T      (psum, contract e over K^T/Q^T chunks)
    P^T      = exp(S^T)   (ACT, no max subtraction - logits < ~60 so exp
                           stays in fp32 range; softmax is shift-invariant)
    out[n,e] = P^T.T @ V  (contract m over all 16 m-chunks in psum)
    den[n]   = P^T.T @ 1  (rides the same stationary weights)
    out     /= den        (DVE reciprocal + per-partition scale)
"""

import numpy as np

import concourse.bass as bass
import concourse.mybir as mybir
import concourse.tile as tile

B, N, D = 4, 2048, 1024
NCORES = 8
P = 128
NQ = N // 2          # query rows per core
KD = D // P          # 8 contraction chunks over d_in
KE = D // P          # 8 chunks over d_out (e)
MC = N // P          # 16 key chunks of 128
MB = N // 512        # 4 key blocks of 512
FP = mybir.dt.float32
FPR = mybir.dt.float32r


def _split_sync_waits(nc, max_waits=1):
    """Walrus codegen on this container accepts at most one sync-wait command
    per instruction; hoist excess waits onto NoOps injected just before the
    instruction on the same engine (engines execute in order, so blocking at
    the NoOp is equivalent)."""
    mb = mybir
    for fn in nc.m.functions:
        for bb in fn.blocks:
            insts = list(bb.instructions)
            new = []
            changed = False
            for inst in insts:
                si = getattr(inst, "sync_info", None)
                if si is not None and si.on_wait and len(si.on_wait) > max_waits:
                    waits = list(si.on_wait)
                    keep = waits[-max_waits:]
                    excess = waits[:-max_waits]
                    for i in range(0, len(excess), max_waits):
                        chunk = excess[i : i + max_waits]
                        nop = mb.InstNoOp(
                            name=f"{inst.name}-sw{i}", ins=[], outs=[],
                            engine=inst.engine,
                        )
                        nop.sync_info = mb.SyncInfo(on_wait=chunk, on_update=[])
                        new.append(nop)
                    inst.sync_info = mb.SyncInfo(
                        on_wait=keep, on_update=list(si.on_update or [])
                    )
                    changed = True
                new.append(inst)
            if changed:
                bb.instructions = new


def _emit_body(nc, tc, rep, params, consts, pools):
    """One full attention computation for this core's shard."""
    XT, Wq, Wk, Wv, OUT = params
    bq_t, bk_t, bv_row, ones_col, ones_row = consts
    v_pool, qt_pool, ktdram, mm_ps, st_ps, out_ps = pools
    MM = nc.tensor.matmul

    KT = ktdram.tile([D, N], FPR, name=f"KT{rep}", tag="KT")
    vt = [v_pool.tile([P, D], FPR, name=f"vt{rep}_{i}", tag="vt") for i in range(MC)]
    qt = [qt_pool.tile([P, NQ], FPR, name=f"qt{rep}_{i}", tag="qt") for i in range(KE)]

    # ---------------- projections ----------------
    with (
        tc.tile_pool(name=f"w{rep}", bufs=16) as w_pool,
        tc.tile_pool(name=f"xt{rep}", bufs=16) as xt_pool,
        tc.tile_pool(name=f"kst{rep}", bufs=4) as kst_pool,
    ):
        # Fused K+V pass: one stream over XT; Wk and Wv both resident.
        # K's 8MB spill-write spreads over ~2x the compute, and XT is read
        # once for both projections (20MB -> 12MB of XT traffic overall).
        wk = [w_pool.tile([P, D], FPR, name=f"wk{rep}_{i}", tag="w") for i in range(KD)]
        for d in range(KD):
            nc.sync.dma_start(wk[d][:], Wk[d * P : (d + 1) * P, :])
        wv = [w_pool.tile([P, D], FPR, name=f"wv{rep}_{i}", tag="w") for i in range(KD)]
        for d in range(KD):
            nc.sync.dma_start(wv[d][:], Wv[d * P : (d + 1) * P, :])
        for mb in range(MB):
            xts = [xt_pool.tile([P, 512], FPR, name=f"xtk{rep}_{i}", tag="xt")
                   for i in range(KD)]
            for d in range(KD):
                nc.sync.dma_start(
                    xts[d][:], XT[d * P : (d + 1) * P, mb * 512 : (mb + 1) * 512]
                )
            for e in range(KE):
                ps = mm_ps.tile([P, 512], FP, name="mm", tag="mm")
                for d in range(KD):
                    MM(ps[:], wk[d][:, e * P : (e + 1) * P], xts[d][:],
                       start=(d == 0), stop=(d == KD - 1))
                st = kst_pool.tile([P, 512], FPR, name="kst", tag="kst")
                nc.vector.tensor_scalar_add(st[:], ps[:], bk_t[:, e : e + 1])
                nc.scalar.dma_start(
                    KT[e * P : (e + 1) * P, mb * 512 : (mb + 1) * 512], st[:]
                )
            for m2 in range(4):
                m = mb * 4 + m2
                for eh in range(2):
                    ps = mm_ps.tile([P, 512], FP, name="mm", tag="mm")
                    for d in range(KD):
                        MM(ps[:], xts[d][:, m2 * P : (m2 + 1) * P],
                           wv[d][:, eh * 512 : (eh + 1) * 512],
                           start=(d == 0), stop=False)
                    MM(ps[:], ones_row[:], bv_row[:, eh * 512 : (eh + 1) * 512],
                       start=False, stop=True)
                    nc.vector.tensor_copy(vt[m][:, eh * 512 : (eh + 1) * 512], ps[:])

        # Q phase: Q^T[e, n] -> SBUF (query rows are XT columns 0..NQ)
        wq = [w_pool.tile([P, D], FPR, name=f"wq{rep}_{i}", tag="w") for i in range(KD)]
        for d in range(KD):
            nc.sync.dma_start(wq[d][:], Wq[d * P : (d + 1) * P, :])
        for mb in range(NQ // 512):
            xts = [xt_pool.tile([P, 512], FPR, name=f"xtq{rep}_{i}", tag="xt")
                   for i in range(KD)]
            for d in range(KD):
                nc.sync.dma_start(
                    xts[d][:], XT[d * P : (d + 1) * P, mb * 512 : (mb + 1) * 512]
                )
            for e in range(KE):
                ps = mm_ps.tile([P, 512], FP, name="mm", tag="mm")
                for d in range(KD):
                    MM(ps[:], wq[d][:, e * P : (e + 1) * P], xts[d][:],
                       start=(d == 0), stop=(d == KD - 1))
                nc.vector.tensor_scalar_add(
                    qt[e][:, mb * 512 : (mb + 1) * 512], ps[:], bq_t[:, e : e + 1]
                )

    # ---------------- attention ----------------
    # P~ is kept for the FULL query range (64KB/partition) so K^T streams
    # from DRAM exactly once: each K^T block scores both query halves.
    with (
        tc.tile_pool(name=f"kts{rep}", bufs=2) as kts_pool,
        tc.tile_pool(name=f"pt{rep}", bufs=MC) as pt_pool,
        tc.tile_pool(name=f"ostage{rep}", bufs=2) as ostage,
        tc.tile_pool(name=f"rec{rep}", bufs=4) as rec_pool,
    ):
        pts = [pt_pool.tile([P, NQ], FPR, name=f"pt{rep}_{i}", tag="pt")
               for i in range(MC)]
        for mb in range(MB):
            ktsb = kts_pool.tile([P, KE, 512], FPR, name=f"kts{rep}", tag="kts")
            nc.scalar.dma_start(
                ktsb[:],
                KT[:, mb * 512 : (mb + 1) * 512].rearrange(
                    "(e p) m -> p e m", p=P
                ),
            )
            kts = [ktsb[:, i, :] for i in range(KE)]
            for m2 in range(4):
                m = mb * 4 + m2
                for nh in range(2):
                    st = st_ps.tile([P, 512], FP, name="st", tag="st")
                    for e in range(KE):
                        MM(st[:], kts[e][:, m2 * P : (m2 + 1) * P],
                           qt[e][:, nh * 512 : (nh + 1) * 512],
                           start=(e == 0), stop=(e == KE - 1))
                    nc.scalar.activation(
                        pts[m][:, nh * 512 : (nh + 1) * 512], st[:],
                        mybir.ActivationFunctionType.Exp,
                    )
        for nh in range(2):
            for ns in range(4):
                o0 = out_ps.tile([P, 512], FP, name="o0", tag="out")
                o1 = out_ps.tile([P, 512], FP, name="o1", tag="out")
                den = mm_ps.tile([P, 512], FP, name="den", tag="mm")
                for m in range(MC):
                    lh = pts[m][:, nh * 512 + ns * P : nh * 512 + (ns + 1) * P]
                    MM(o0[:], lh, vt[m][:, 0:512],
                       start=(m == 0), stop=(m == MC - 1))
                    MM(o1[:], lh, vt[m][:, 512:1024],
                       start=(m == 0), stop=(m == MC - 1))
                    MM(den[:, 0:2], lh, ones_col[:],
                       start=(m == 0), stop=(m == MC - 1))
                rec = rec_pool.tile([P, 1], FP, name="rec", tag="rec")
                nc.vector.reciprocal(rec[:], den[:, 0:1])
                ost = ostage.tile([P, D], FP, name="ost", tag="ost")
                nc.vector.tensor_scalar_mul(ost[:, 0:512], o0[:], rec[:])
                nc.vector.tensor_scalar_mul(ost[:, 512:1024], o1[:], rec[:])
                nrow = nh * 512 + ns * P
                nc.scalar.dma_start(OUT[nrow : nrow + P, :], ost[:])


def build_bass(split=True, reps=1):
    nc = bass.Bass()
    XT = nc.declare_dram_parameter("XT", [D, N], FPR, isOutput=False)
    Wq = nc.declare_dram_parameter("Wq", [D, D], FPR, isOutput=False)
    Wk = nc.declare_dram_parameter("Wk", [D, D], FPR, isOutput=False)
    Wv = nc.declare_dram_parameter("Wv", [D, D], FPR, isOutput=False)
    BQ = nc.declare_dram_parameter("bq_t", [P, KE], FP, isOutput=False)
    BK = nc.declare_dram_parameter("bk_t", [P, KE], FP, isOutput=False)
    BV = nc.declare_dram_parameter("bv_row", [1, D], FPR, isOutput=False)
    ONESC = nc.declare_dram_parameter("ones_col", [P, 2], FPR, isOutput=False)
    ONESR = nc.declare_dram_parameter("ones_row", [1, P], FPR, isOutput=False)
    OUT = nc.declare_dram_parameter("OUT", [NQ, D], FP, isOutput=True)

    with tile.TileContext(nc) as tc:
        with (
            tc.tile_pool(name="misc", bufs=1) as misc,
            tc.tile_pool(name="vt", bufs=MC) as v_pool,
            tc.tile_pool(name="qt", bufs=KE) as qt_pool,
            tc.tile_pool(name="ktdram", bufs=1, space="DRAM") as ktdram,
            tc.tile_pool(name="mmps", bufs=2, space="PSUM") as mm_ps,
            tc.tile_pool(name="stps", bufs=2, space="PSUM") as st_ps,
            tc.tile_pool(name="outps", bufs=4, space="PSUM") as out_ps,
        ):
            bq_t = misc.tile([P, KE], FP, tag="bq")
            bk_t = misc.tile([P, KE], FP, tag="bk")
            bv_row = misc.tile([1, D], FPR, tag="bv")
            ones_col = misc.tile([P, 2], FPR, tag="onc")
            ones_row = misc.tile([1, P], FPR, tag="onr")
            nc.sync.dma_start(bq_t[:], BQ[:])
            nc.sync.dma_start(bk_t[:], BK[:])
            nc.sync.dma_start(bv_row[:], BV[:])
            nc.sync.dma_start(ones_col[:], ONESC[:])
            nc.sync.dma_start(ones_row[:], ONESR[:])

            params = (XT, Wq, Wk, Wv, OUT)
            consts = (bq_t, bk_t, bv_row, ones_col, ones_row)
            pools = (v_pool, qt_pool, ktdram, mm_ps, st_ps, out_ps)
            for rep in range(reps):
                _emit_body(nc, tc, rep, params, consts, pools)

    if split:
        _split_sync_waits(nc)
    return nc


_CACHE = {}


def _get_runner(reps=1, donate=True):
    """Compile once; return fn(in_maps) -> list[dict] running SPMD on 8 cores.

    reps>1 repeats the whole kernel body inside the NEFF (used for timing:
    slope over reps isolates per-body device time from dispatch overhead).
    """
    key = (reps, donate)
    if key in _CACHE:
        return _CACHE[key]

    import jax
    from jax.experimental.shard_map import shard_map
    from jax.sharding import Mesh, PartitionSpec

    from concourse import bass2jax

    nc = build_bass(reps=reps)
    bass2jax.install_neuronx_cc_hook()

    partition_name = (
        nc.partition_id_tensor.name if nc.partition_id_tensor else None
    )
    in_names, out_names, out_avals, zero_outs = [], [], [], []
    for alloc in nc.m.functions[0].allocations:
        if not isinstance(alloc, mybir.MemoryLocationSet):
            continue
        name = alloc.memorylocations[0].name
        if alloc.kind == "ExternalInput":
            if name != partition_name:
                in_names.append(name)
        elif alloc.kind == "ExternalOutput":
            shape = tuple(alloc.tensor_shape)
            dtype = mybir.dt.np(alloc.dtype)
            out_names.append(name)
            out_avals.append(jax.core.ShapedArray(shape, dtype))
            zero_outs.append(np.zeros(shape, dtype))
    n_params = len(in_names)
    n_outs = len(out_avals)
    all_in_names = list(in_names) + list(out_names)
    if partition_name is not None:
        all_in_names.append(partition_name)
    donate_idx = tuple(range(n_params, n_params + n_outs))

    def _body(*args):
        operands = list(args)
        if partition_name is not None:
            operands.append(bass2jax.partition_id_tensor())
        outs = bass2jax._bass_exec_p.bind(
            *operands,
            out_avals=tuple(out_avals),
            in_names=tuple(all_in_names),
            out_names=tuple(out_names),
            lowering_input_output_aliases=(),
            sim_require_finite=True,
            sim_require_nnan=True,
            nc=nc,
        )
        return tuple(outs)

    devices = jax.devices()[:NCORES]
    mesh = Mesh(np.asarray(devices), ("core",))
    in_specs = (PartitionSpec("core"),) * (n_params + n_outs)
    out_specs = (PartitionSpec("core"),) * n_outs
    sharded = jax.jit(
        shard_map(
            _body, mesh=mesh, in_specs=in_specs, out_specs=out_specs,
            check_rep=False,
        ),
        donate_argnums=donate_idx if donate else (),
        keep_unused=True,
    )

    def run(in_maps):
        import jax as _jax

        per_core = [[np.asarray(m[name]) for name in in_names] for m in in_maps]
        concat_in = [
            np.concatenate([per_core[c][i] for c in range(NCORES)], axis=0)
            for i in range(n_params)
        ]
        concat_zero = [np.concatenate([z] * NCORES, axis=0) for z in zero_outs]
        outs = sharded(*concat_in, *concat_zero)
        outs = [np.asarray(o) for o in _jax.block_until_ready(outs)]
        results = []
        for c in range(NCORES):
            r = {}
            for i, name in enumerate(out_names):
                d0 = out_avals[i].shape[0]
                r[name] = outs[i][c * d0 : (c + 1) * d0]
            results.append(r)
        return results

    run.sharded = sharded
    run.n_params = n_params
    run.in_names = in_names
    run.zero_outs = zero_outs
    _CACHE[key] = run
    return run


def _in_maps(X, Wq, bq, Wk, bk, Wv, bv):
    X = np.asarray(X, np.float32)
    maps = []
    bq_t = np.ascontiguousarray(np.asarray(bq, np.float32).reshape(KE, P).T)
    bk_t = np.ascontiguousarray(np.asarray(bk, np.float32).reshape(KE, P).T)
    bv_row = np.ascontiguousarray(np.asarray(bv, np.float32).reshape(1, D))
    Wq = np.ascontiguousarray(np.asarray(Wq, np.float32))
    Wk = np.ascontiguousarray(np.asarray(Wk, np.float32))
    Wv = np.ascontiguousarray(np.asarray(Wv, np.float32))
    for c in range(NCORES):
        b, h = c // 2, c % 2
        Xb = X[b]
        rows = np.concatenate(
            [Xb[h * NQ : (h + 1) * NQ], Xb[(1 - h) * NQ : (2 - h) * NQ]], axis=0
        )
        XT = np.ascontiguousarray(rows.T)
        maps.append(
            dict(XT=XT, Wq=Wq, Wk=Wk, Wv=Wv, bq_t=bq_t, bk_t=bk_t,
                 bv_row=bv_row, ones_col=np.ones((P, 2), np.float32),
                 ones_row=np.ones((1, P), np.float32))
        )
    return maps


def kernel(X, Wq, bq, Wk, bk, Wv, bv):
    run = _get_runner()
    results = run(_in_maps(X, Wq, bq, Wk, bk, Wv, bv))
    out = np.empty((B, N, D), np.float32)
    for c in range(NCORES):
        b, h = c // 2, c % 2
        out[b, h * NQ : (h + 1) * NQ, :] = results[c]["OUT"]
    return out
